# revision 2
# baseline (speedup 1.0000x reference)
"""Trainium2 Bass kernel for nn_Apply_Mask (topk_masking). v15.

Per (batch, channel) slice of shape 32x32: find the argmax location, build
a clipped (2*half+1)^2 box around it, S = 1 - box, lam = 1024/sum(S), and
out = (T != 0) ? x * S * lam : x.

Sharding: data-parallel over the 32768 b*c slices; core i takes slices
[4096*i, 4096*(i+1)). Per-core layout: partition p holds slices
[32p, 32p+32); tile t = slice 32p+t at free offset t*1024.

Engine split (v15): DVE runs the two mandatory full-data 1-port passes
(reduce_max XY for per-tile maxima -> needles, FIND_INDEX8 over the
8-tile window for the argmax position) plus the small box/lambda math and
the final select STT (u = (q==0) * xa -> bf16). GpSimd (shares DVE's 2nd
SBUF port, so it runs while DVE is in 1-port argmax ops) builds the
binary box mask q via one broadcast outer product per 8-tile group.
ScalarE (own ports) produces xa = bf16(a * x) with the per-tile scale a =
1 + sel*(lam-1) riding the cast. DMA: f32 in (4 x 4 MiB), bf16 out.
"""
import sys

for _p in ("/opt/trn_rl_repo",):
    if _p not in sys.path:
        sys.path.insert(0, _p)

import numpy as np

import concourse.bass as bass
import concourse.tile as tile
from concourse import bacc, mybir
from concourse.bass_utils import run_bass_kernel_spmd

P = 128
NT = 32
H = W = 32
HW = H * W
N_CORES = 8
SLICES_PER_CORE = P * NT

GT = 8                 # tiles per group
NG = NT // GT          # 4 groups
GSZ = GT * HW          # 8192 elems per group per partition

f32 = mybir.dt.float32
bf16 = mybir.dt.bfloat16
u16 = mybir.dt.uint16
Alu = mybir.AluOpType
Act = mybir.ActivationFunctionType
AxX = mybir.AxisListType.X
AxXY = mybir.AxisListType.XY

_cached = {}


def _build(half: int):
    nc = bacc.Bacc("TRN2", target_bir_lowering=False, debug=False,
                   num_devices=N_CORES)
    x_in = nc.dram_tensor("x", [P, NT * HW], f32, kind="ExternalInput").ap()
    sel_in = nc.dram_tensor("sel", [P, NT], f32, kind="ExternalInput").ap()
    io_in = nc.dram_tensor("io32", [P, 32], f32, kind="ExternalInput").ap()
    out_d = nc.dram_tensor("out", [P, NT * HW], bf16, kind="ExternalOutput").ap()

    with tile.TileContext(nc) as tc:
        from contextlib import ExitStack
        with ExitStack() as ctx:
            xpool = ctx.enter_context(tc.tile_pool(name="xp", bufs=2))
            qpool = ctx.enter_context(tc.tile_pool(name="qp", bufs=2))
            apool = ctx.enter_context(tc.tile_pool(name="ap", bufs=2))
            mpool = ctx.enter_context(tc.tile_pool(name="mp", bufs=2))
            small = ctx.enter_context(tc.tile_pool(name="small", bufs=1))

            selp = small.tile([P, NT], f32)
            nc.sync.dma_start(selp[:], sel_in)
            io32 = small.tile([P, 32], f32)
            nc.sync.dma_start(io32[:], io_in)

            xc = []
            for g in range(NG):
                t_ = xpool.tile([P, GSZ], f32, name=f"x{g}", tag="x")
                nc.sync.dma_start(t_[:], x_in[:, g * GSZ:(g + 1) * GSZ])
                xc.append(t_)

            nselp = small.tile([P, NT], f32)
            nc.vector.tensor_scalar(nselp[:], selp[:], -1.0, 1.0, Alu.mult, Alu.add)

            tmax = small.tile([P, NT], f32)
            idxg = small.tile([P, NT], u16)
            a32 = small.tile([P, NT], f32)

            for g in range(NG):
                gs = slice(g * GT, (g + 1) * GT)
                xg = xc[g]
                x4 = xg[:].rearrange("p (t h w) -> p t h w", t=GT, h=H, w=W)

                # ---- argmax: per-tile max (needles) + flat index ----
                nc.vector.tensor_reduce(tmax[:, gs], x4, axis=AxXY, op=Alu.max)
                nc.vector.max_index(idxg[:, gs], tmax[:, gs], xg[:])

                # ---- box bounds + lambda ----
                mhw_u = small.tile([P, 2 * GT], u16, name=f"mhwu{g}", tag="mhwu")
                nc.vector.tensor_scalar(mhw_u[:, 0:GT], idxg[:, gs], 5, 31,
                                        Alu.logical_shift_right, Alu.bitwise_and)
                nc.vector.tensor_scalar(mhw_u[:, GT:2 * GT], idxg[:, gs], 31, None,
                                        Alu.bitwise_and)
                mhw = small.tile([P, 2 * GT], f32, name=f"mhw{g}", tag="mhw")
                nc.vector.tensor_copy(mhw[:], mhw_u[:])
                b1 = small.tile([P, 2 * GT], f32, name=f"b1{g}", tag="b1")
                b2 = small.tile([P, 2 * GT], f32, name=f"b2{g}", tag="b2")
                nc.vector.tensor_scalar(b1[:], mhw[:], float(half), 0.0,
                                        Alu.subtract, Alu.max)
                nc.vector.tensor_scalar(b2[:], mhw[:], float(half), float(H - 1),
                                        Alu.add, Alu.min)
                e1 = small.tile([P, 2 * GT], f32, name=f"e1{g}", tag="e1")
                nc.vector.scalar_tensor_tensor(e1[:], b1[:], -1.0, b2[:],
                                               Alu.mult, Alu.add)
                nc.vector.tensor_scalar(e1[:], e1[:], 1.0, None, Alu.add)
                area = small.tile([P, GT], f32, name=f"area{g}", tag="area")
                nc.vector.tensor_tensor(area[:], e1[:, 0:GT], e1[:, GT:2 * GT],
                                        Alu.mult)
                nc.vector.tensor_scalar(area[:], area[:], -1.0, float(HW),
                                        Alu.mult, Alu.add)
                rec = small.tile([P, GT], f32, name=f"rec{g}", tag="rec")
                nc.vector.reciprocal(rec[:], area[:])
                asel = small.tile([P, GT], f32, name=f"asel{g}", tag="asel")
                nc.vector.scalar_tensor_tensor(asel[:], rec[:], float(HW),
                                               selp[:, gs], Alu.mult, Alu.mult)
                nc.vector.tensor_tensor(a32[:, gs], asel[:], nselp[:, gs], Alu.add)

                # ---- row/col membership vectors ----
                iob = io32[:, None, :].broadcast_to([P, 2 * GT, 32])
                lo = small.tile([P, 2 * GT, 32], f32, name=f"lo{g}", tag="lo")
                hi = small.tile([P, 2 * GT, 32], f32, name=f"hi{g}", tag="hi")
                nc.vector.tensor_tensor(
                    lo[:], iob, b1[:, :, None].broadcast_to([P, 2 * GT, 32]),
                    Alu.is_ge)
                nc.vector.tensor_tensor(
                    hi[:], iob, b2[:, :, None].broadcast_to([P, 2 * GT, 32]),
                    Alu.is_gt)
                rc = small.tile([P, 2 * GT, 32], f32, name=f"rc{g}", tag="rc")
                nc.gpsimd.tensor_tensor(rc[:], lo[:], hi[:], Alu.subtract)
                nc.gpsimd.tensor_tensor(
                    rc[:, 0:GT], rc[:, 0:GT],
                    selp[:, gs, None].broadcast_to([P, GT, 32]), Alu.mult)

                # ---- box mask (binary) via one outer product ----
                q = qpool.tile([P, GT, H, W], f32, name=f"q{g}", tag="q")
                nc.gpsimd.tensor_tensor(
                    q[:],
                    rc[:, 0:GT, :, None].broadcast_to([P, GT, H, W]),
                    rc[:, GT:2 * GT, None, :].broadcast_to([P, GT, H, W]),
                    Alu.mult)

                # ---- xa = bf16(a * x) on ScalarE (per-tile scale) ----
                xa = apool.tile([P, GSZ], bf16, name=f"xa{g}", tag="xa")
                for j in range(GT):
                    t = g * GT + j
                    nc.scalar.activation(
                        xa[:, j * HW:(j + 1) * HW], xg[:, j * HW:(j + 1) * HW],
                        Act.Copy, bias=0.0, scale=a32[:, t:t + 1])

                # ---- select: u = (q == 0) * xa  -> bf16 out ----
                u = mpool.tile([P, GSZ], bf16, name=f"u{g}", tag="u")
                nc.vector.scalar_tensor_tensor(
                    u[:], q[:].rearrange("p t h w -> p (t h w)"), 0.0, xa[:],
                    Alu.is_equal, Alu.mult)
                nc.sync.dma_start(out_d[:, g * GSZ:(g + 1) * GSZ], u[:])

    nc.compile()
    return nc


def _get_nc(half: int):
    if half not in _cached:
        _cached[half] = _build(half)
    return _cached[half]


def _shard_inputs(x, T):
    xf = np.ascontiguousarray(x, dtype=np.float32).reshape(-1, HW)
    sel = (np.asarray(T).reshape(-1) != 0).astype(np.float32)
    io32 = np.tile(np.arange(32, dtype=np.float32), (P, 1))
    in_maps = []
    for i in range(N_CORES):
        lo = i * SLICES_PER_CORE
        hi = lo + SLICES_PER_CORE
        in_maps.append({
            "x": np.ascontiguousarray(xf[lo:hi].reshape(P, NT * HW)),
            "sel": np.ascontiguousarray(sel[lo:hi].reshape(P, NT)),
            "io32": io32,
        })
    return in_maps


def run(inputs, trace=False, **kw):
    x = inputs["x"]
    T = inputs["T"]
    drop_block = int(np.asarray(inputs["drop_block"]))
    half = drop_block // 2
    b, c, h, w = x.shape
    assert (h, w) == (H, W) and b * c == N_CORES * SLICES_PER_CORE, \
        f"kernel hardcoded for (128,256,32,32); got {x.shape}"

    nc = _get_nc(half)
    in_maps = _shard_inputs(x, T)
    res = run_bass_kernel_spmd(nc, in_maps, core_ids=list(range(N_CORES)),
                               trace=trace, **kw)
    parts = [np.asarray(res.results[i]["out"]).astype(np.float32)
              .reshape(SLICES_PER_CORE, HW)
             for i in range(N_CORES)]
    out = np.concatenate(parts, axis=0).reshape(b, c, h, w)
    return out, res


def kernel(**inputs) -> np.ndarray:
    out, _ = run(inputs, trace=False)
    return out


# revision 4
# speedup vs baseline: 1.0866x; 1.0866x over previous
"""Trainium2 Bass kernel for nn_Apply_Mask (topk_masking). v16.

Per (batch, channel) slice of shape 32x32: find the argmax location, build
a clipped (2*half+1)^2 box around it, S = 1 - box, lam = 1024/sum(S), and
out = (T != 0) ? x * S * lam : x.

Sharding: data-parallel over the 32768 b*c slices; core i takes slices
[4096*i, 4096*(i+1)). Per-core layout: partition p holds slices
[32p, 32p+32); tile t = slice 32p+t at free offset t*1024.

Engine split (v16): DVE runs the two mandatory full-data 1-port passes
(reduce_max XY for per-tile maxima -> FIND_INDEX8 needles; FIND_INDEX8
over the 8-tile window for the argmax position), the small box/lambda
math, and the final select as a contiguous bf16 TT multiply u = S * xa
(2x DVE mode). GpSimd (shares DVE's 2nd SBUF port; scheduled under DVE's
1-port argmax phases) builds the binary box mask q with one broadcast
outer product per group. ScalarE (own ports) produces xa = bf16(a * x)
(per-tile scale a = 1 + sel*(lam-1) rides the cast) and S = bf16(1 - q)
(Copy, scale=-1, bias=1). sel is folded into the row bounds (unselected
slices get an empty box), so q = 0 and a = 1 there -> out = x.
Emission is pipelined: select(g-1) is issued after argmax(g) so DVE never
blocks on the GpSimd/ScalarE round trip.
"""
import sys

for _p in ("/opt/trn_rl_repo",):
    if _p not in sys.path:
        sys.path.insert(0, _p)

import numpy as np

import concourse.bass as bass
import concourse.tile as tile
from concourse import bacc, mybir
from concourse.bass_utils import run_bass_kernel_spmd

P = 128
NT = 32
H = W = 32
HW = H * W
N_CORES = 8
SLICES_PER_CORE = P * NT

GT = 8                 # tiles per group
NG = NT // GT          # 4 groups
GSZ = GT * HW          # 8192 elems per group per partition

f32 = mybir.dt.float32
bf16 = mybir.dt.bfloat16
u16 = mybir.dt.uint16
Alu = mybir.AluOpType
Act = mybir.ActivationFunctionType
AxXY = mybir.AxisListType.XY

_cached = {}


def _build(half: int):
    nc = bacc.Bacc("TRN2", target_bir_lowering=False, debug=False,
                   num_devices=N_CORES)
    x_in = nc.dram_tensor("x", [P, NT * HW], f32, kind="ExternalInput").ap()
    sel_in = nc.dram_tensor("sel", [P, NT], f32, kind="ExternalInput").ap()
    io_in = nc.dram_tensor("io32", [P, 32], f32, kind="ExternalInput").ap()
    out_d = nc.dram_tensor("out", [P, NT * HW], bf16, kind="ExternalOutput").ap()

    with tile.TileContext(nc) as tc:
        from contextlib import ExitStack
        with ExitStack() as ctx:
            xpool = ctx.enter_context(tc.tile_pool(name="xp", bufs=2))
            qpool = ctx.enter_context(tc.tile_pool(name="qp", bufs=2))
            apool = ctx.enter_context(tc.tile_pool(name="ap", bufs=2))
            spool = ctx.enter_context(tc.tile_pool(name="sp", bufs=2))
            small = ctx.enter_context(tc.tile_pool(name="small", bufs=1))

            selp = small.tile([P, NT], f32)
            nc.sync.dma_start(selp[:], sel_in)
            io32 = small.tile([P, 32], f32)
            nc.sync.dma_start(io32[:], io_in)

            xc = []
            for g in range(NG):
                t_ = xpool.tile([P, GSZ], f32, name=f"x{g}", tag="x")
                nc.sync.dma_start(t_[:], x_in[:, g * GSZ:(g + 1) * GSZ])
                xc.append(t_)

            nselp = small.tile([P, NT], f32)
            nc.vector.tensor_scalar(nselp[:], selp[:], -1.0, 1.0, Alu.mult, Alu.add)

            tmax = small.tile([P, NT], f32)
            idxg = small.tile([P, NT], u16)
            a32 = small.tile([P, NT], f32)
            xas = {}
            ss = {}

            def emit_group(g):
                gs = slice(g * GT, (g + 1) * GT)
                xg = xc[g]
                x4 = xg[:].rearrange("p (t h w) -> p t h w", t=GT, h=H, w=W)

                # ---- DVE argmax: per-tile max (needles) + flat index ----
                nc.vector.tensor_reduce(tmax[:, gs], x4, axis=AxXY, op=Alu.max)
                nc.vector.max_index(idxg[:, gs], tmax[:, gs], xg[:])

                # ---- DVE box bounds + lambda (all [P,8/16] smalls) ----
                mhw_u = small.tile([P, 2 * GT], u16, name=f"mhwu{g}", tag="mhwu")
                nc.vector.tensor_scalar(mhw_u[:, 0:GT], idxg[:, gs], 5, 31,
                                        Alu.logical_shift_right, Alu.bitwise_and)
                nc.vector.tensor_scalar(mhw_u[:, GT:2 * GT], idxg[:, gs], 31, None,
                                        Alu.bitwise_and)
                mhw = small.tile([P, 2 * GT], f32, name=f"mhw{g}", tag="mhw")
                nc.vector.tensor_copy(mhw[:], mhw_u[:])
                b1 = small.tile([P, 2 * GT], f32, name=f"b1{g}", tag="b1")
                b2 = small.tile([P, 2 * GT], f32, name=f"b2{g}", tag="b2")
                nc.vector.tensor_scalar(b1[:], mhw[:], float(half), 0.0,
                                        Alu.subtract, Alu.max)
                nc.vector.tensor_scalar(b2[:], mhw[:], float(half), float(H - 1),
                                        Alu.add, Alu.min)
                e1 = small.tile([P, 2 * GT], f32, name=f"e1{g}", tag="e1")
                nc.vector.scalar_tensor_tensor(e1[:], b1[:], -1.0, b2[:],
                                               Alu.mult, Alu.add)
                nc.vector.tensor_scalar(e1[:], e1[:], 1.0, None, Alu.add)
                area = small.tile([P, GT], f32, name=f"area{g}", tag="area")
                nc.vector.tensor_tensor(area[:], e1[:, 0:GT], e1[:, GT:2 * GT],
                                        Alu.mult)
                nc.vector.tensor_scalar(area[:], area[:], -1.0, float(HW),
                                        Alu.mult, Alu.add)
                rec = small.tile([P, GT], f32, name=f"rec{g}", tag="rec")
                nc.vector.reciprocal(rec[:], area[:])
                asel = small.tile([P, GT], f32, name=f"asel{g}", tag="asel")
                nc.vector.scalar_tensor_tensor(asel[:], rec[:], float(HW),
                                               selp[:, gs], Alu.mult, Alu.mult)
                nc.vector.tensor_tensor(a32[:, gs], asel[:], nselp[:, gs], Alu.add)
                # unselected slices get an empty row range (beyond h=31):
                # b1_row += 99*(1-sel), b2_row += 99*(1-sel)
                nc.vector.scalar_tensor_tensor(b1[:, 0:GT], nselp[:, gs], 99.0,
                                               b1[:, 0:GT], Alu.mult, Alu.add)
                nc.vector.scalar_tensor_tensor(b2[:, 0:GT], nselp[:, gs], 99.0,
                                               b2[:, 0:GT], Alu.mult, Alu.add)

                # ---- DVE row/col membership vectors rc [P,16,32] ----
                iob = io32[:, None, :].broadcast_to([P, 2 * GT, 32])
                lo = small.tile([P, 2 * GT, 32], f32, name=f"lo{g}", tag="lo")
                hi = small.tile([P, 2 * GT, 32], f32, name=f"hi{g}", tag="hi")
                nc.vector.tensor_tensor(
                    lo[:], iob, b1[:, :, None].broadcast_to([P, 2 * GT, 32]),
                    Alu.is_ge)
                nc.vector.tensor_tensor(
                    hi[:], iob, b2[:, :, None].broadcast_to([P, 2 * GT, 32]),
                    Alu.is_gt)
                rc = small.tile([P, 2 * GT, 32], f32, name=f"rc{g}", tag="rc")
                nc.vector.scalar_tensor_tensor(rc[:], hi[:], -1.0, lo[:],
                                               Alu.mult, Alu.add)

                # ---- GpSimd: binary box mask via one outer product ----
                q = qpool.tile([P, GT, H, W], f32, name=f"q{g}", tag="q")
                nc.gpsimd.tensor_tensor(
                    q[:],
                    rc[:, 0:GT, :, None].broadcast_to([P, GT, H, W]),
                    rc[:, GT:2 * GT, None, :].broadcast_to([P, GT, H, W]),
                    Alu.mult)

                # ---- ScalarE: xa = bf16(a * x), S = bf16(1 - q) ----
                xa = apool.tile([P, GSZ], bf16, name=f"xa{g}", tag="xa")
                for j in range(GT):
                    t = g * GT + j
                    nc.scalar.activation(
                        xa[:, j * HW:(j + 1) * HW], xg[:, j * HW:(j + 1) * HW],
                        Act.Copy, bias=0.0, scale=a32[:, t:t + 1])
                s_ = spool.tile([P, GSZ], bf16, name=f"s{g}", tag="s")
                nc.scalar.activation(
                    s_[:], q[:].rearrange("p t h w -> p (t h w)"),
                    Act.Copy, bias=1.0, scale=-1.0)
                xas[g] = xa
                ss[g] = s_

            def emit_select(g):
                # ---- DVE select: u = S * xa (bf16 2x), in-place into xa ----
                xa, s_ = xas[g], ss[g]
                nc.vector.tensor_tensor(xa[:], s_[:], xa[:], Alu.mult)
                nc.sync.dma_start(out_d[:, g * GSZ:(g + 1) * GSZ], xa[:])

            for g in range(NG):
                emit_group(g)
                if g >= 1:
                    emit_select(g - 1)
            emit_select(NG - 1)

    nc.compile()
    return nc


def _get_nc(half: int):
    if half not in _cached:
        _cached[half] = _build(half)
    return _cached[half]


def _shard_inputs(x, T):
    xf = np.ascontiguousarray(x, dtype=np.float32).reshape(-1, HW)
    sel = (np.asarray(T).reshape(-1) != 0).astype(np.float32)
    io32 = np.tile(np.arange(32, dtype=np.float32), (P, 1))
    in_maps = []
    for i in range(N_CORES):
        lo = i * SLICES_PER_CORE
        hi = lo + SLICES_PER_CORE
        in_maps.append({
            "x": np.ascontiguousarray(xf[lo:hi].reshape(P, NT * HW)),
            "sel": np.ascontiguousarray(sel[lo:hi].reshape(P, NT)),
            "io32": io32,
        })
    return in_maps


def run(inputs, trace=False, **kw):
    x = inputs["x"]
    T = inputs["T"]
    drop_block = int(np.asarray(inputs["drop_block"]))
    half = drop_block // 2
    b, c, h, w = x.shape
    assert (h, w) == (H, W) and b * c == N_CORES * SLICES_PER_CORE, \
        f"kernel hardcoded for (128,256,32,32); got {x.shape}"

    nc = _get_nc(half)
    in_maps = _shard_inputs(x, T)
    res = run_bass_kernel_spmd(nc, in_maps, core_ids=list(range(N_CORES)),
                               trace=trace, **kw)
    parts = [np.asarray(res.results[i]["out"]).astype(np.float32)
              .reshape(SLICES_PER_CORE, HW)
             for i in range(N_CORES)]
    out = np.concatenate(parts, axis=0).reshape(b, c, h, w)
    return out, res


def kernel(**inputs) -> np.ndarray:
    out, _ = run(inputs, trace=False)
    return out


# revision 10
# speedup vs baseline: 1.4571x; 1.3410x over previous
"""Trainium2 Bass kernel for nn_Apply_Mask (topk_masking). v17.

Per (batch, channel) slice of shape 32x32: find the argmax location, build
a clipped (2*half+1)^2 box around it, S = 1 - box, lam = 1024/sum(S), and
out = (T != 0) ? x * S * lam : x.

Sharding: data-parallel over the 32768 b*c slices; core i takes slices
[4096*i, 4096*(i+1)). Per-core layout: partition p holds slices
[32p, 32p+32); tile t = slice 32p+t at free offset t*1024.

Architecture (v17, all-DVE): GpSimd fully blocks concurrent DVE
instructions (shared SBUF port lock), so it is not used at all. DVE does
everything dataplane: per-tile maxima via segmented reduce_max (XY), the
argmax position via FIND_INDEX8 over each 8-tile window (needles are the
reduce output, no copies), the box/lambda smalls, bf16 row/col factors
A = a*(1 - in_row), B = a*(1 - in_col) (a = 1 + sel*(lam-1); sel is
folded into the row bounds so unselected slices get an empty box and
a = 1), the mask m = max(A2_bc, B_bc) as a bf16 TT in 2x DVE mode (A is
pairwise-duplicated and addressed with 4-free-dim APs so the innermost
pair is contiguous - stride-0-innermost would force 1x), and the apply
u = xb * m as a contiguous bf16 TT (2x). ScalarE (independent ports)
provides xb = bf16(x) early and the A pair-duplication. The mask/apply of
group g runs during group g+1's argmax so the ScalarE round trip never
stalls DVE. DMA: f32 in (8 x 2 MiB), bf16 out (4 x 2 MiB).
"""
import sys

for _p in ("/opt/trn_rl_repo",):
    if _p not in sys.path:
        sys.path.insert(0, _p)

import numpy as np

import concourse.bass as bass
import concourse.tile as tile
from concourse import bacc, mybir
from concourse.bass_utils import run_bass_kernel_spmd

P = 128
NT = 32
H = W = 32
HW = H * W
N_CORES = 8
SLICES_PER_CORE = P * NT

GT = 8                 # tiles per group
NG = NT // GT          # 4 groups
GSZ = GT * HW          # 8192 elems per group per partition

f32 = mybir.dt.float32
bf16 = mybir.dt.bfloat16
u16 = mybir.dt.uint16
Alu = mybir.AluOpType
Act = mybir.ActivationFunctionType
AxXY = mybir.AxisListType.XY

_cached = {}


def _build(half: int):
    nc = bacc.Bacc("TRN2", target_bir_lowering=False, debug=False,
                   num_devices=N_CORES)
    x_in = nc.dram_tensor("x", [P, NT * HW], f32, kind="ExternalInput").ap()
    sel_in = nc.dram_tensor("sel", [P, NT], f32, kind="ExternalInput").ap()
    io_in = nc.dram_tensor("io32", [P, 32], f32, kind="ExternalInput").ap()
    out_d = nc.dram_tensor("out", [P, NT * HW], bf16, kind="ExternalOutput").ap()

    with tile.TileContext(nc) as tc:
        from contextlib import ExitStack
        with ExitStack() as ctx:
            xpool = ctx.enter_context(tc.tile_pool(name="xp", bufs=2))
            bpool = ctx.enter_context(tc.tile_pool(name="bp", bufs=2))
            mpool = ctx.enter_context(tc.tile_pool(name="mp", bufs=2))
            small = ctx.enter_context(tc.tile_pool(name="small", bufs=2))

            xc = []
            for g in range(NG):
                t_ = xpool.tile([P, GSZ], f32, name=f"x{g}", tag="x")
                # split chunk DMAs so the first reduce can start earlier
                nparts = 4 if g == 0 else 2
                for k in range(nparts):
                    lo_ = g * GSZ + k * GSZ // nparts
                    nc.sync.dma_start(
                        t_[:, k * GSZ // nparts:(k + 1) * GSZ // nparts],
                        x_in[:, lo_:lo_ + GSZ // nparts])
                xc.append(t_)

            selp = small.tile([P, NT], f32)
            nc.sync.dma_start(selp[:], sel_in)
            io32 = small.tile([P, 32], f32)
            nc.sync.dma_start(io32[:], io_in)

            nselp = small.tile([P, NT], f32)
            nc.vector.tensor_scalar(nselp[:], selp[:], -1.0, 1.0, Alu.mult, Alu.add)

            tmax = small.tile([P, NT], f32)
            idxg = small.tile([P, NT], u16)
            st = {}

            def emit_argmax(g):
                gs = slice(g * GT, (g + 1) * GT)
                xg = xc[g]
                # per-tile maxima (= FI needles), split to chase the DMA
                nparts = 4 if g == 0 else 2
                tp = GT // nparts
                for h_ in range(nparts):
                    hs = slice(g * GT + h_ * tp, g * GT + (h_ + 1) * tp)
                    x4 = xg[:, h_ * tp * HW:(h_ + 1) * tp * HW].rearrange(
                        "p (t h w) -> p t h w", t=tp, h=H, w=W)
                    nc.vector.tensor_reduce(tmax[:, hs], x4, axis=AxXY, op=Alu.max)
                nc.vector.max_index(idxg[:, gs], tmax[:, gs], xg[:])

            def emit_mask(g):
                gs = slice(g * GT, (g + 1) * GT)
                xg = xc[g]

                # ---- box bounds + lambda ([P,8/16] smalls) ----
                mhw_u = small.tile([P, 2 * GT], u16, name=f"mhwu{g}", tag="mhwu")
                nc.vector.tensor_scalar(mhw_u[:, 0:GT], idxg[:, gs], 5, 31,
                                        Alu.logical_shift_right, Alu.bitwise_and)
                nc.vector.tensor_scalar(mhw_u[:, GT:2 * GT], idxg[:, gs], 31, None,
                                        Alu.bitwise_and)
                mhw = small.tile([P, 2 * GT], f32, name=f"mhw{g}", tag="mhw")
                nc.vector.tensor_copy(mhw[:], mhw_u[:])
                # unselected slices: push the box beyond h=31 (empty row range)
                nc.vector.scalar_tensor_tensor(mhw[:, 0:GT], nselp[:, gs], 99.0,
                                               mhw[:, 0:GT], Alu.mult, Alu.add)
                b1 = small.tile([P, 2 * GT], f32, name=f"b1{g}", tag="b1")
                b2p = small.tile([P, 2 * GT], f32, name=f"b2p{g}", tag="b2p")
                nc.vector.tensor_scalar(b1[:], mhw[:], float(half), 0.0,
                                        Alu.subtract, Alu.max)
                # b2p = b2 + 1 = min(mhw + half + 1, 32): turns (io > b2) into
                # is_ge(io, b2p) and makes extents b2p - b1 directly
                nc.vector.tensor_scalar(b2p[:], mhw[:], float(half + 1), float(H),
                                        Alu.add, Alu.min)
                e1 = small.tile([P, 2 * GT], f32, name=f"e1{g}", tag="e1")
                nc.vector.scalar_tensor_tensor(e1[:], b1[:], -1.0, b2p[:],
                                               Alu.mult, Alu.add)
                area = small.tile([P, GT], f32, name=f"area{g}", tag="area")
                nc.vector.tensor_tensor(area[:], e1[:, 0:GT], e1[:, GT:2 * GT],
                                        Alu.mult)
                nc.vector.tensor_scalar(area[:], area[:], -1.0, float(HW),
                                        Alu.mult, Alu.add)
                rec = small.tile([P, GT], f32, name=f"rec{g}", tag="rec")
                nc.vector.reciprocal(rec[:], area[:])
                asel = small.tile([P, GT], f32, name=f"asel{g}", tag="asel")
                nc.vector.scalar_tensor_tensor(asel[:], rec[:], float(HW),
                                               selp[:, gs], Alu.mult, Alu.mult)
                a_ = small.tile([P, GT], f32, name=f"a{g}", tag="a")
                nc.vector.tensor_tensor(a_[:], asel[:], nselp[:, gs], Alu.add)

                # ---- membership vectors inb [P,16,32] in {0,1} ----
                iob = io32[:, None, :].broadcast_to([P, 2 * GT, 32])
                lo = small.tile([P, 2 * GT, 32], f32, name=f"lo{g}", tag="lo")
                hi = small.tile([P, 2 * GT, 32], f32, name=f"hi{g}", tag="hi")
                nc.vector.tensor_tensor(
                    lo[:], iob, b1[:, :, None].broadcast_to([P, 2 * GT, 32]),
                    Alu.is_ge)
                nc.vector.tensor_tensor(
                    hi[:], iob, b2p[:, :, None].broadcast_to([P, 2 * GT, 32]),
                    Alu.is_ge)
                inb = small.tile([P, 2 * GT, 32], f32, name=f"inb{g}", tag="inb")
                nc.vector.scalar_tensor_tensor(inb[:], hi[:], -1.0, lo[:],
                                               Alu.mult, Alu.add)

                # ---- A/B factors (bf16): value a outside box range, 0 inside
                ab = bpool.tile([P, 2 * GT, 32], bf16, name=f"ab{g}", tag="ab")
                a_bc = a_[:, :, None].broadcast_to([P, GT, 32])
                nc.vector.scalar_tensor_tensor(
                    ab[:, 0:GT], inb[:, 0:GT], 0.0, a_bc, Alu.is_equal, Alu.mult)
                nc.vector.scalar_tensor_tensor(
                    ab[:, GT:2 * GT], inb[:, GT:2 * GT], 0.0, a_bc,
                    Alu.is_equal, Alu.mult)

                # ---- ScalarE: pairwise-dup of the row factor ----
                a2 = bpool.tile([P, GT, 32, 2], bf16, name=f"a2{g}", tag="a2")
                nc.scalar.activation(
                    a2[:], ab[:, 0:GT, :, None].broadcast_to([P, GT, 32, 2]),
                    Act.Copy, bias=0.0, scale=1.0)

                # ---- ScalarE: xb = bf16(x) ----
                xb = mpool.tile([P, GSZ], bf16, name=f"xb{g}", tag="xb")
                nc.scalar.activation(xb[:], xg[:], Act.Copy, bias=0.0, scale=1.0)
                st[g] = (a2, ab, xb)

            def emit_apply(g):
                a2, ab, xb = st[g]
                # m = max(A2_bc, B_pairs_bc): bf16 TT in 2x mode (4-dim APs)
                m = mpool.tile([P, GT, 32, 16, 2], bf16, name=f"m{g}", tag="m")
                bp = ab[:, GT:2 * GT].rearrange("p t (w2 two) -> p t w2 two",
                                                w2=16, two=2)
                nc.vector.tensor_tensor(
                    m[:],
                    a2[:, :, :, None, :].broadcast_to([P, GT, 32, 16, 2]),
                    bp[:, :, None, :, :].broadcast_to([P, GT, 32, 16, 2]),
                    Alu.max)
                # u = xb * m (contiguous bf16 TT, 2x), in place into xb
                nc.vector.tensor_tensor(
                    xb[:], xb[:],
                    m[:].rearrange("p t h w2 two -> p (t h w2 two)"), Alu.mult)
                nc.sync.dma_start(out_d[:, g * GSZ:(g + 1) * GSZ], xb[:])

            emit_argmax(0)
            emit_mask(0)
            for g in range(1, NG):
                emit_argmax(g)
                emit_apply(g - 1)
                emit_mask(g)
            emit_apply(NG - 1)

    nc.compile()
    return nc


def _get_nc(half: int):
    if half not in _cached:
        _cached[half] = _build(half)
    return _cached[half]


def _shard_inputs(x, T):
    xf = np.ascontiguousarray(x, dtype=np.float32).reshape(-1, HW)
    sel = (np.asarray(T).reshape(-1) != 0).astype(np.float32)
    io32 = np.tile(np.arange(32, dtype=np.float32), (P, 1))
    in_maps = []
    for i in range(N_CORES):
        lo = i * SLICES_PER_CORE
        hi = lo + SLICES_PER_CORE
        in_maps.append({
            "x": np.ascontiguousarray(xf[lo:hi].reshape(P, NT * HW)),
            "sel": np.ascontiguousarray(sel[lo:hi].reshape(P, NT)),
            "io32": io32,
        })
    return in_maps


def run(inputs, trace=False, **kw):
    x = inputs["x"]
    T = inputs["T"]
    drop_block = int(np.asarray(inputs["drop_block"]))
    half = drop_block // 2
    b, c, h, w = x.shape
    assert (h, w) == (H, W) and b * c == N_CORES * SLICES_PER_CORE, \
        f"kernel hardcoded for (128,256,32,32); got {x.shape}"

    nc = _get_nc(half)
    in_maps = _shard_inputs(x, T)
    res = run_bass_kernel_spmd(nc, in_maps, core_ids=list(range(N_CORES)),
                               trace=trace, **kw)
    parts = [np.asarray(res.results[i]["out"]).astype(np.float32)
              .reshape(SLICES_PER_CORE, HW)
             for i in range(N_CORES)]
    out = np.concatenate(parts, axis=0).reshape(b, c, h, w)
    return out, res


def kernel(**inputs) -> np.ndarray:
    out, _ = run(inputs, trace=False)
    return out


# revision 12
# speedup vs baseline: 1.4808x; 1.0163x over previous
"""Trainium2 Bass kernel for nn_Apply_Mask (topk_masking). v17.

Per (batch, channel) slice of shape 32x32: find the argmax location, build
a clipped (2*half+1)^2 box around it, S = 1 - box, lam = 1024/sum(S), and
out = (T != 0) ? x * S * lam : x.

Sharding: data-parallel over the 32768 b*c slices; core i takes slices
[4096*i, 4096*(i+1)). Per-core layout: partition p holds slices
[32p, 32p+32); tile t = slice 32p+t at free offset t*1024.

Architecture (v17, all-DVE): GpSimd fully blocks concurrent DVE
instructions (shared SBUF port lock), so it is not used at all. DVE does
everything dataplane: per-tile maxima via segmented reduce_max (XY), the
argmax position via FIND_INDEX8 over each 8-tile window (needles are the
reduce output, no copies), the box/lambda smalls, bf16 row/col factors
A = a*(1 - in_row), B = a*(1 - in_col) (a = 1 + sel*(lam-1); sel is
folded into the row bounds so unselected slices get an empty box and
a = 1), the mask m = max(A2_bc, B_bc) as a bf16 TT in 2x DVE mode (A is
pairwise-duplicated and addressed with 4-free-dim APs so the innermost
pair is contiguous - stride-0-innermost would force 1x), and the apply
u = xb * m as a contiguous bf16 TT (2x). ScalarE (independent ports)
provides xb = bf16(x) early and the A pair-duplication. The mask/apply of
group g runs during group g+1's argmax so the ScalarE round trip never
stalls DVE. DMA: f32 in (8 x 2 MiB), bf16 out (4 x 2 MiB).
"""
import sys

for _p in ("/opt/trn_rl_repo",):
    if _p not in sys.path:
        sys.path.insert(0, _p)

import numpy as np

import concourse.bass as bass
import concourse.tile as tile
from concourse import bacc, mybir
from concourse.bass_utils import run_bass_kernel_spmd

P = 128
NT = 32
H = W = 32
HW = H * W
N_CORES = 8
SLICES_PER_CORE = P * NT

GT = 8                 # tiles per group
NG = NT // GT          # 4 groups
GSZ = GT * HW          # 8192 elems per group per partition

f32 = mybir.dt.float32
bf16 = mybir.dt.bfloat16
u16 = mybir.dt.uint16
Alu = mybir.AluOpType
Act = mybir.ActivationFunctionType
AxXY = mybir.AxisListType.XY

_cached = {}


def _build(half: int):
    nc = bacc.Bacc("TRN2", target_bir_lowering=False, debug=False,
                   num_devices=N_CORES)
    x_in = nc.dram_tensor("x", [P, NT * HW], f32, kind="ExternalInput").ap()
    sel_in = nc.dram_tensor("sel", [P, NT], f32, kind="ExternalInput").ap()
    io_in = nc.dram_tensor("io32", [P, 32], f32, kind="ExternalInput").ap()
    out_d = nc.dram_tensor("out", [P, NT * HW], bf16, kind="ExternalOutput").ap()

    with tile.TileContext(nc) as tc:
        from contextlib import ExitStack
        with ExitStack() as ctx:
            xpool = ctx.enter_context(tc.tile_pool(name="xp", bufs=2))
            bpool = ctx.enter_context(tc.tile_pool(name="bp", bufs=2))
            mpool = ctx.enter_context(tc.tile_pool(name="mp", bufs=2))
            small = ctx.enter_context(tc.tile_pool(name="small", bufs=2))

            xc = []
            for g in range(NG):
                t_ = xpool.tile([P, GSZ], f32, name=f"x{g}", tag="x")
                # split chunk DMAs so the first reduce can start earlier
                nparts = 4 if g == 0 else 2
                for k in range(nparts):
                    lo_ = g * GSZ + k * GSZ // nparts
                    nc.sync.dma_start(
                        t_[:, k * GSZ // nparts:(k + 1) * GSZ // nparts],
                        x_in[:, lo_:lo_ + GSZ // nparts])
                xc.append(t_)

            selp = small.tile([P, NT], f32)
            nc.sync.dma_start(selp[:], sel_in)
            io32 = small.tile([P, 32], f32)
            nc.sync.dma_start(io32[:], io_in)

            nselp = small.tile([P, NT], f32)
            nc.vector.tensor_scalar(nselp[:], selp[:], -1.0, 1.0, Alu.mult, Alu.add)

            tmax = small.tile([P, NT], f32)
            idxg = small.tile([P, NT], u16)
            st = {}

            def emit_argmax(g):
                gs = slice(g * GT, (g + 1) * GT)
                xg = xc[g]
                # per-tile maxima (= FI needles), split to chase the DMA
                nparts = 4 if g == 0 else 2
                tp = GT // nparts
                for h_ in range(nparts):
                    hs = slice(g * GT + h_ * tp, g * GT + (h_ + 1) * tp)
                    x4 = xg[:, h_ * tp * HW:(h_ + 1) * tp * HW].rearrange(
                        "p (t h w) -> p t h w", t=tp, h=H, w=W)
                    nc.vector.tensor_reduce(tmax[:, hs], x4, axis=AxXY, op=Alu.max)
                nc.vector.max_index(idxg[:, gs], tmax[:, gs], xg[:])

            def emit_mask(g):
                gs = slice(g * GT, (g + 1) * GT)
                xg = xc[g]

                # ---- box bounds + lambda ([P,8/16] smalls) ----
                mhw_u = small.tile([P, 2 * GT], u16, name=f"mhwu{g}", tag="mhwu")
                nc.vector.tensor_scalar(mhw_u[:, 0:GT], idxg[:, gs], 5, 31,
                                        Alu.logical_shift_right, Alu.bitwise_and)
                nc.vector.tensor_scalar(mhw_u[:, GT:2 * GT], idxg[:, gs], 31, None,
                                        Alu.bitwise_and)
                mhw = small.tile([P, 2 * GT], f32, name=f"mhw{g}", tag="mhw")
                nc.vector.tensor_copy(mhw[:], mhw_u[:])
                # unselected slices: push the box beyond h=31 (empty row range)
                nc.vector.scalar_tensor_tensor(mhw[:, 0:GT], nselp[:, gs], 99.0,
                                               mhw[:, 0:GT], Alu.mult, Alu.add)
                b1 = small.tile([P, 2 * GT], f32, name=f"b1{g}", tag="b1")
                b2p = small.tile([P, 2 * GT], f32, name=f"b2p{g}", tag="b2p")
                nc.vector.tensor_scalar(b1[:], mhw[:], float(half), 0.0,
                                        Alu.subtract, Alu.max)
                # b2p = b2 + 1 = min(mhw + half + 1, 32): turns (io > b2) into
                # is_ge(io, b2p) and makes extents b2p - b1 directly
                nc.vector.tensor_scalar(b2p[:], mhw[:], float(half + 1), float(H),
                                        Alu.add, Alu.min)
                e1 = small.tile([P, 2 * GT], f32, name=f"e1{g}", tag="e1")
                nc.vector.scalar_tensor_tensor(e1[:], b1[:], -1.0, b2p[:],
                                               Alu.mult, Alu.add)
                area = small.tile([P, GT], f32, name=f"area{g}", tag="area")
                nc.vector.tensor_tensor(area[:], e1[:, 0:GT], e1[:, GT:2 * GT],
                                        Alu.mult)
                nc.vector.tensor_scalar(area[:], area[:], -1.0, float(HW),
                                        Alu.mult, Alu.add)
                rec = small.tile([P, GT], f32, name=f"rec{g}", tag="rec")
                nc.vector.reciprocal(rec[:], area[:])
                asel = small.tile([P, GT], f32, name=f"asel{g}", tag="asel")
                nc.vector.scalar_tensor_tensor(asel[:], rec[:], float(HW),
                                               selp[:, gs], Alu.mult, Alu.mult)
                a_ = small.tile([P, GT], f32, name=f"a{g}", tag="a")
                nc.vector.tensor_tensor(a_[:], asel[:], nselp[:, gs], Alu.add)

                # ---- membership vectors inb [P,16,32] in {0,1} ----
                iob = io32[:, None, :].broadcast_to([P, 2 * GT, 32])
                lo = small.tile([P, 2 * GT, 32], f32, name=f"lo{g}", tag="lo")
                hi = small.tile([P, 2 * GT, 32], f32, name=f"hi{g}", tag="hi")
                nc.vector.tensor_tensor(
                    lo[:], iob, b1[:, :, None].broadcast_to([P, 2 * GT, 32]),
                    Alu.is_ge)
                nc.vector.tensor_tensor(
                    hi[:], iob, b2p[:, :, None].broadcast_to([P, 2 * GT, 32]),
                    Alu.is_ge)
                inb = small.tile([P, 2 * GT, 32], f32, name=f"inb{g}", tag="inb")
                nc.vector.scalar_tensor_tensor(inb[:], hi[:], -1.0, lo[:],
                                               Alu.mult, Alu.add)

                # ---- A/B factors (bf16): value a outside box range, 0 inside
                ab = bpool.tile([P, 2 * GT, 32], bf16, name=f"ab{g}", tag="ab")
                a_bc = a_[:, :, None].broadcast_to([P, GT, 32])
                nc.vector.scalar_tensor_tensor(
                    ab[:, 0:GT], inb[:, 0:GT], 0.0, a_bc, Alu.is_equal, Alu.mult)
                nc.vector.scalar_tensor_tensor(
                    ab[:, GT:2 * GT], inb[:, GT:2 * GT], 0.0, a_bc,
                    Alu.is_equal, Alu.mult)

                # ---- ScalarE: pairwise-dup of the row factor ----
                a2 = bpool.tile([P, GT, 32, 2], bf16, name=f"a2{g}", tag="a2")
                nc.scalar.activation(
                    a2[:], ab[:, 0:GT, :, None].broadcast_to([P, GT, 32, 2]),
                    Act.Copy, bias=0.0, scale=1.0)

                # ---- ScalarE: xb = bf16(x) ----
                xb = mpool.tile([P, GSZ], bf16, name=f"xb{g}", tag="xb")
                nc.scalar.activation(xb[:], xg[:], Act.Copy, bias=0.0, scale=1.0)
                st[g] = (a2, ab, xb)

            def emit_apply(g, nparts=1):
                a2, ab, xb = st[g]
                # m = max(A2_bc, B_pairs_bc): bf16 TT in 2x mode (4-dim APs)
                m = mpool.tile([P, GT, 32, 16, 2], bf16, name=f"m{g}", tag="m")
                bp = ab[:, GT:2 * GT].rearrange("p t (w2 two) -> p t w2 two",
                                                w2=16, two=2)
                tp = GT // nparts
                for k in range(nparts):
                    ts_ = slice(k * tp, (k + 1) * tp)
                    nc.vector.tensor_tensor(
                        m[:, ts_],
                        a2[:, ts_, :, None, :].broadcast_to([P, tp, 32, 16, 2]),
                        bp[:, ts_, None, :, :].broadcast_to([P, tp, 32, 16, 2]),
                        Alu.max)
                    # u = xb * m (contiguous bf16 TT, 2x), in place into xb
                    nc.vector.tensor_tensor(
                        xb[:, k * tp * HW:(k + 1) * tp * HW],
                        xb[:, k * tp * HW:(k + 1) * tp * HW],
                        m[:, ts_].rearrange("p t h w2 two -> p (t h w2 two)"),
                        Alu.mult)
                    nc.sync.dma_start(
                        out_d[:, g * GSZ + k * tp * HW:
                              g * GSZ + (k + 1) * tp * HW],
                        xb[:, k * tp * HW:(k + 1) * tp * HW])

            emit_argmax(0)
            emit_mask(0)
            for g in range(1, NG):
                emit_argmax(g)
                emit_apply(g - 1)
                emit_mask(g)
            emit_apply(NG - 1, nparts=2)

    nc.compile()
    return nc


def _get_nc(half: int):
    if half not in _cached:
        _cached[half] = _build(half)
    return _cached[half]


def _shard_inputs(x, T):
    xf = np.ascontiguousarray(x, dtype=np.float32).reshape(-1, HW)
    sel = (np.asarray(T).reshape(-1) != 0).astype(np.float32)
    io32 = np.tile(np.arange(32, dtype=np.float32), (P, 1))
    in_maps = []
    for i in range(N_CORES):
        lo = i * SLICES_PER_CORE
        hi = lo + SLICES_PER_CORE
        in_maps.append({
            "x": np.ascontiguousarray(xf[lo:hi].reshape(P, NT * HW)),
            "sel": np.ascontiguousarray(sel[lo:hi].reshape(P, NT)),
            "io32": io32,
        })
    return in_maps


def run(inputs, trace=False, **kw):
    x = inputs["x"]
    T = inputs["T"]
    drop_block = int(np.asarray(inputs["drop_block"]))
    half = drop_block // 2
    b, c, h, w = x.shape
    assert (h, w) == (H, W) and b * c == N_CORES * SLICES_PER_CORE, \
        f"kernel hardcoded for (128,256,32,32); got {x.shape}"

    nc = _get_nc(half)
    in_maps = _shard_inputs(x, T)
    res = run_bass_kernel_spmd(nc, in_maps, core_ids=list(range(N_CORES)),
                               trace=trace, **kw)
    parts = [np.asarray(res.results[i]["out"]).astype(np.float32)
              .reshape(SLICES_PER_CORE, HW)
             for i in range(N_CORES)]
    out = np.concatenate(parts, axis=0).reshape(b, c, h, w)
    return out, res


def kernel(**inputs) -> np.ndarray:
    out, _ = run(inputs, trace=False)
    return out
